# revision 33
# baseline (speedup 1.0000x reference)
"""Causal single-head attention (S=4096, D=1024, fp32) on 8 TRN2 NeuronCores.

Uniform interleaved-row causal scheme, rank-major streaming.
- Core c owns q rows c::8: its 4 q-tiles of 128 rows span global ranges
  [1024t, 1024(t+1)) and need key-tiles tau < 8(t+1) -- identical structure on
  every core (SPMD-uniform), yet ~half the score/AV work of the full rectangle
  is skipped by causality. Diagonal masks are 8 host-provided 128x128 tiles.
- K/V projections sharded 8-way (rank r projects key-tiles {r,r+8,r+16,r+24});
  shared via 3 chip-wide AllGathers: K fp8 (4MB), V bf16 as two vd-halves
  (4MB each) so the vd0 A@V sweep overlaps the vd1 gather. The ~25-50us launch
  barrier is runtime-fixed (first collective executes ~58-85us in); the pre-AG
  window is filled with q-projection and a LOCAL attention pass on the core's
  own 4 key-tiles (kloc/vloc, no AG dependency). Own-rank stream tiles are
  zeroed via a per-tile exp bias input (-1e9), keeping the stream uniform.
- kvin stores ride the scalar queue (fire as soon as projection copies drain);
  gathered K/V stream back per RANK-block (512KB contiguous DMAs, 4KB lines).
- Scores run fp8 DoubleRow ([128,2,x] operand pairs over 256-wide contraction);
  exp folds the 1/sqrt(D) scale; softmax denominators ride the A@V sweep as
  16-wide ones-matmuls sharing the pt LDWEIGHTS; per-q-tile normalization is
  emitted inline at each accumulator's closing pair in the final rank.
- Projections and A@V stay bf16: fp8 V fails the 2e-2 gate (0.035 measured in
  simulation) and fp8 q/k-projection doubles the error for no critical-path
  gain. Measured: ~190-205us vs 318-331us baseline, rel err 0.0070.
"""

import numpy as np
import ml_dtypes

import concourse.bacc as bacc
import concourse.tile as tile
from concourse import mybir
from concourse.bass_utils import run_bass_kernel_spmd

S = 4096
D = 1024
NCORES = 8
P = 128
DC = 8          # contraction blocks of 128 in D
NT = 4          # q-tiles per core (128 rows each)
NKT = 32        # key tiles of 128 globally
BF = mybir.dt.bfloat16
F32 = mybir.dt.float32
FP8 = mybir.dt.float8e4
EXP = mybir.ActivationFunctionType.Exp
DR = mybir.MatmulPerfMode.DoubleRow
ALL8 = [[0, 1, 2, 3, 4, 5, 6, 7]]

bf16 = ml_dtypes.bfloat16


def build_nc(dbg=False):
    nc = bacc.Bacc(None, target_bir_lowering=False, debug=False)
    if dbg:
        d_pt = nc.declare_dram_parameter("d_pt", [4, P, 512], BF, isOutput=True)
        d_sums = nc.declare_dram_parameter("d_sums", [P, 64], F32, isOutput=True)

    xq = nc.declare_dram_parameter("xqT", [D, 512], BF, isOutput=False)
    xk = nc.declare_dram_parameter("xkT", [D, 512], BF, isOutput=False)
    xv = nc.declare_dram_parameter("xvT", [D, 512], BF, isOutput=False)
    wq = nc.declare_dram_parameter("wqT", [D, D], BF, isOutput=False)
    wk = nc.declare_dram_parameter("wkT", [D, D], BF, isOutput=False)
    wv = nc.declare_dram_parameter("wvT", [D, D], BF, isOutput=False)
    msk = nc.declare_dram_parameter("masks", [9, P, P], BF, isOutput=False)
    bias_p = nc.declare_dram_parameter("biasv", [P, NKT], F32, isOutput=False)
    out = nc.declare_dram_parameter("out", [512, D], F32, isOutput=True)

    kvin_k = nc.dram_tensor("kvin_k", [P, NT, DC, P], FP8)
    kvout_k = nc.dram_tensor("kvout_k", [NCORES * P, NT, DC, P], FP8, addr_space="Shared")
    VW = [640, 384]   # vd widths gathered per phase (640 = 512 + av5's 128)
    kvin_v = [nc.dram_tensor(f"kvin_v{h}", [P, NT, VW[h]], BF) for h in range(2)]
    kvout_v = [nc.dram_tensor(f"kvout_v{h}", [NCORES * P, NT, VW[h]], BF, addr_space="Shared")
               for h in range(2)]

    with tile.TileContext(nc) as tc:
        with (
            tc.tile_pool(name="persist", bufs=1) as persist,
            tc.tile_pool(name="wp", bufs=16) as wp,
            tc.tile_pool(name="xp", bufs=16) as xp,
            tc.tile_pool(name="ptp", bufs=1) as ptp,
            tc.tile_pool(name="ktp", bufs=4) as ktp,
            tc.tile_pool(name="vtp", bufs=4) as vtp,
            tc.tile_pool(name="outp", bufs=4) as outp,
            tc.tile_pool(name="sps", bufs=2, space="PSUM") as sps,
            tc.tile_pool(name="avs", bufs=1, space="PSUM") as avs,
            tc.tile_pool(name="sums", bufs=1, space="PSUM") as sums_pool,
        ):
            ones = persist.tile([P, 16], BF, tag="ones", name="ones")
            nc.vector.memset(ones[:], 1.0)

            m_t = [persist.tile([P, P], BF, tag=f"m{d}", name=f"m{d}") for d in range(9)]
            bias_t = persist.tile([P, NKT], F32, tag="bias", name="bias")

            qT = persist.tile([P, DC, 512], FP8, tag="qT", name="qT")
            kloc = persist.tile([P, NT, DC, P], FP8, tag="kloc", name="kloc")
            vloc = [persist.tile([P, NT, VW[h]], BF, tag=f"vloc{h}", name=f"vloc{h}")
                    for h in range(2)]

            # ---- K projection: kT blocks [outdim 128, 512 keys] -> fp8 ----
            wk_t = [wp.tile([P, D], BF, tag="w", name=f"wk{d}") for d in range(DC)]
            xk_t = [xp.tile([P, 512], BF, tag="x", name=f"xk{d}") for d in range(DC)]
            for d in range(DC):
                nc.sync.dma_start(out=xk_t[d][:], in_=xk[d * P:(d + 1) * P, :])
                nc.sync.dma_start(out=wk_t[d][:], in_=wk[d * P:(d + 1) * P, :])
            for ob in range(DC):
                ps = sps.tile([P, 512], F32, tag="sp", name="ppk")
                for d in range(DC):
                    nc.tensor.matmul(
                        ps[:], lhsT=wk_t[d][:, ob * P:(ob + 1) * P], rhs=xk_t[d][:],
                        start=(d == 0), stop=(d == DC - 1),
                    )
                for s in range(NT):
                    nc.scalar.copy(kloc[:, s, ob, :], ps[:, s * P:(s + 1) * P])

            # ---- V projection: v blocks [keys 128, 512 vd] -> bf16 ----
            wv_t = [wp.tile([P, D], BF, tag="w", name=f"wv{d}") for d in range(DC)]
            xv_t = [xp.tile([P, 512], BF, tag="x", name=f"xv{d}") for d in range(DC)]
            for d in range(DC):
                nc.sync.dma_start(out=xv_t[d][:], in_=xv[d * P:(d + 1) * P, :])
                nc.sync.dma_start(out=wv_t[d][:], in_=wv[d * P:(d + 1) * P, :])
            for h in range(2):
                for s in range(NT):
                    ps = sps.tile([P, 512], F32, tag="sp", name="ppv")
                    for d in range(DC):
                        nc.tensor.matmul(
                            ps[:], lhsT=xv_t[d][:, s * P:(s + 1) * P],
                            rhs=wv_t[d][:, h * 512:(h + 1) * 512],
                            start=(d == 0), stop=(d == DC - 1),
                        )
                    if h == 0:
                        nc.scalar.copy(vloc[0][:, s, 0:512], ps[:])
                    else:
                        nc.scalar.copy(vloc[0][:, s, 512:640], ps[:, 0:P])
                        nc.scalar.copy(vloc[1][:, s, :], ps[:, P:512])

            # ---- Q projection -> fp8 qT [128, cb, 512] ----
            wq_t = [wp.tile([P, D], BF, tag="w", name=f"wq{d}") for d in range(DC)]
            xq_t = [xp.tile([P, 512], BF, tag="x", name=f"xq{d}") for d in range(DC)]
            for d in range(DC):
                nc.sync.dma_start(out=xq_t[d][:], in_=xq[d * P:(d + 1) * P, :])
                nc.sync.dma_start(out=wq_t[d][:], in_=wq[d * P:(d + 1) * P, :])
            # constants arriving on sync after the big loads
            for dd in range(9):
                nc.sync.dma_start(out=m_t[dd][:], in_=msk[dd, :, :])
            nc.sync.dma_start(out=bias_t[:], in_=bias_p[:])
            for ob in range(DC):
                ps = sps.tile([P, 512], F32, tag="sp", name="ppq")
                for d in range(DC):
                    nc.tensor.matmul(
                        ps[:], lhsT=wq_t[d][:, ob * P:(ob + 1) * P], rhs=xq_t[d][:],
                        start=(d == 0), stop=(d == DC - 1),
                    )
                nc.scalar.copy(qT[:, ob, :], ps[:])

            # ---- collectives: triggers queue before the barrier ends ----
            # kvin stores ride the scalar queue: they fire right after the
            # projection copies drain instead of behind sync's 9MB of loads
            nc.scalar.dma_start(out=kvin_k[:], in_=kloc[:])
            nc.gpsimd.collective_compute(
                "AllGather", mybir.AluOpType.bypass, replica_groups=ALL8,
                ins=[kvin_k[:].opt()], outs=[kvout_k[:].opt()],
            )
            for h in range(2):
                nc.scalar.dma_start(out=kvin_v[h][:], in_=vloc[h][:])
                nc.gpsimd.collective_compute(
                    "AllGather", mybir.AluOpType.bypass, replica_groups=ALL8,
                    ins=[kvin_v[h][:].opt()], outs=[kvout_v[h][:].opt()],
                )

            sums_bank = sums_pool.tile([P, 64], F32, tag="sums", name="sums")
            av = {}
            for t in range(NT):
                av[t] = avs.tile([P, 512], F32, tag=f"av{t}", name=f"av{t}")
            # vd 512:640 accumulator: one bank, 4 q-tile regions (h0 only)
            av5 = avs.tile([P, 512], F32, tag="av5", name="av5")

            def scores_tile(kt_ap, rhs_hi, tmin, mask, bias_ap, ptag):
                """DR scores + exp + diag mask for one key tile; returns pt."""
                N = rhs_hi - tmin * P
                ps = sps.tile([P, 512], F32, tag="sp", name="sps")
                for dd in range(4):
                    nc.tensor.matmul(
                        ps[:, :N], lhsT=kt_ap[:, 2 * dd:2 * dd + 2, :],
                        rhs=qT[:, 2 * dd:2 * dd + 2, tmin * P:rhs_hi],
                        start=(dd == 0), stop=(dd == 3), perf_mode=DR,
                    )
                p = ptp.tile([P, 512], BF, tag=ptag, name=ptag)
                if bias_ap is None:
                    nc.scalar.activation(p[:, :N], ps[:, :N], EXP, scale=0.03125)
                else:
                    nc.scalar.activation(p[:, :N], ps[:, :N], EXP, scale=0.03125,
                                         bias=bias_ap)
                nc.vector.tensor_mul(p[:, 0:P], p[:, 0:P], mask[:])
                return p

            def sums_av(p, s, t, vt_ap, h, first, last):
                """piggybacked sums + AV MM(s) for pair (s, t); h0 adds av5"""
                pslice = p[:, (t - s) * P:(t - s + 1) * P]
                if h == 0:
                    nc.tensor.matmul(
                        sums_bank[:, t * 16:(t + 1) * 16], lhsT=pslice, rhs=ones[:],
                        start=first and t == 0, stop=last,
                        skip_group_check=True,
                    )
                    nc.tensor.matmul(
                        av[t][:, 0:512], lhsT=pslice, rhs=vt_ap[:, 0:512],
                        start=first, stop=last, skip_group_check=True,
                    )
                    nc.tensor.matmul(
                        av5[:, t * P:(t + 1) * P], lhsT=pslice, rhs=vt_ap[:, 512:640],
                        start=first and t == 0, stop=last, skip_group_check=True,
                    )
                else:
                    nc.tensor.matmul(
                        av[t][:, 0:384], lhsT=pslice, rhs=vt_ap,
                        start=first, stop=last, skip_group_check=True,
                    )

            # ---- local pass: own 4 key-tiles, no AG dependency ----
            ptl = {}
            for s in range(NT):
                ptl[s] = scores_tile(kloc[:, s, :, :], 512, s, m_t[8], None, f"ptl{s}")
                if dbg and s < 2:
                    nc.gpsimd.dma_start(out=d_pt[s, :, :], in_=ptl[s][:])
                for t in range(s, NT):
                    sums_av(ptl[s], s, t, vloc[0][:, s, :], 0, first=(s == 0),
                            last=False)

            # ---- stream scores, rank-major (own rank zeroed via bias) ----
            pt = {}
            ktb = {}
            for r in range(NCORES + 1):
                if r < NCORES:
                    kb = ktp.tile([P, NT, DC, P], FP8, tag="ktb", name="ktb")
                    ktb[r] = kb
                    nc.sync.dma_start(out=kb[:], in_=kvout_k[r * P:(r + 1) * P, :, :, :])
                if r < 1:
                    continue
                rr = r - 1
                for s in range(NT):
                    tau = 8 * s + rr
                    pt[tau] = scores_tile(
                        ktb[rr][:, s, :, :], 512, s, m_t[rr],
                        bias_t[:, tau:tau + 1], f"pt{tau}")

            # ---- A@V vd-half sweeps, rank-major ----
            for h in range(2):
                if h == 1:
                    # re-alloc accumulators (banks freed after h=0 normalize)
                    for t in range(NT):
                        av[t] = avs.tile([P, 512], F32, tag=f"av{t}", name=f"av{t}")
                    # local pairs first: vloc-only, runs during AG_v1 wait
                    for s in range(NT):
                        for t in range(s, NT):
                            sums_av(ptl[s], s, t, vloc[1][:, s, :], 1,
                                    first=(s == 0), last=False)
                for r in range(NCORES):
                    vb = vtp.tile([P, NT, 640], BF, tag="vtb", name="vtb")
                    nc.sync.dma_start(
                        out=vb[:, :, 0:VW[h]], in_=kvout_v[h][r * P:(r + 1) * P, :, :])
                    final = r == NCORES - 1
                    for s in range(NT):
                        tau = 8 * s + r
                        for t in range(s, NT):
                            # in the final rank, pair (s=t, t) is av[t]'s last MM:
                            # close the group and normalize inline
                            sums_av(pt[tau], s, t, vb[:, s, 0:VW[h]], h, first=False,
                                    last=(final and s == t))
                            if final and s == t:
                                ot = outp.tile([P, 640], F32, tag="ot", name="ot")
                                if h == 0:
                                    if dbg and t == NT - 1:
                                        dsb = outp.tile([P, 64], F32, tag="dsb", name="dsb")
                                        nc.vector.tensor_copy(dsb[:], sums_bank[:])
                                        nc.gpsimd.dma_start(out=d_sums[:], in_=dsb[:])
                                    rc = outp.tile([P, 1], F32, tag=f"rec{t}", name=f"rec{t}")
                                    if t == 0:
                                        rec = {}
                                    rec[t] = rc
                                    nc.vector.reciprocal(rc[:], sums_bank[:, t * 16:t * 16 + 1])
                                    nc.vector.tensor_scalar_mul(
                                        ot[:, 0:512], av[t][:], rec[t][:])
                                    nc.vector.tensor_scalar_mul(
                                        ot[:, 512:640], av5[:, t * P:(t + 1) * P], rec[t][:])
                                    nc.sync.dma_start(
                                        out=out[t * P:(t + 1) * P, 0:640], in_=ot[:])
                                else:
                                    nc.vector.tensor_scalar_mul(
                                        ot[:, 0:384], av[t][:, 0:384], rec[t][:])
                                    nc.sync.dma_start(
                                        out=out[t * P:(t + 1) * P, 640:1024],
                                        in_=ot[:, 0:384])
    return nc


_CACHE = {}


def _get_nc():
    if "nc" not in _CACHE:
        nc = build_nc()
        nc.compile()
        _CACHE["nc"] = nc
    return _CACHE["nc"]


def build_in_maps(inputs):
    x_q = np.asarray(inputs["encodings_for_q"], dtype=np.float32)
    x_k = np.asarray(inputs["encodings_for_k"], dtype=np.float32)
    x_v = np.asarray(inputs["encodings_for_v"], dtype=np.float32)
    W_q = np.asarray(inputs["W_q"], dtype=np.float32)
    W_k = np.asarray(inputs["W_k"], dtype=np.float32)
    W_v = np.asarray(inputs["W_v"], dtype=np.float32)

    # no 1/sqrt(D) folding: the exp activation applies scale=1/32
    wqT = np.ascontiguousarray(W_q.T).astype(bf16)
    wkT = np.ascontiguousarray(W_k.T).astype(bf16)
    wvT = np.ascontiguousarray(W_v.T).astype(bf16)

    ik = np.arange(P)[:, None]
    iq = np.arange(P)[None, :]

    in_maps = []
    for c in range(NCORES):
        rows = np.arange(c, S, NCORES)
        keys = np.concatenate([np.arange(P * (8 * s + c), P * (8 * s + c) + P)
                               for s in range(NT)])
        masks = np.stack([(128 * d + ik <= 8 * iq + c) for d in range(8)]
                         + [(128 * c + ik <= 8 * iq + c)])  # slot 8 = own diag (d=c)
        biasv = np.zeros((P, NKT), dtype=np.float32)
        biasv[:, [c, c + 8, c + 16, c + 24]] = -1e9   # zero own-rank stream tiles
        in_maps.append(dict(
            xqT=np.ascontiguousarray(x_q[rows].T).astype(bf16),
            xkT=np.ascontiguousarray(x_k[keys].T).astype(bf16),
            xvT=np.ascontiguousarray(x_v[keys].T).astype(bf16),
            wqT=wqT, wkT=wkT, wvT=wvT,
            masks=masks.astype(bf16),
            biasv=biasv,
        ))
    return in_maps


def kernel(**inputs):
    nc = _get_nc()
    in_maps = build_in_maps(inputs)
    res = run_bass_kernel_spmd(nc, in_maps, list(range(NCORES)))
    full = np.empty((S, D), dtype=np.float32)
    for c in range(NCORES):
        full[c::NCORES] = np.asarray(res.results[c]["out"], dtype=np.float32)
    return full


# revision 34
# speedup vs baseline: 1.0879x; 1.0879x over previous
"""Causal single-head attention (S=4096, D=1024, fp32) on 8 TRN2 NeuronCores.

Uniform interleaved-row causal scheme, rank-major streaming.
- Core c owns q rows c::8: its 4 q-tiles of 128 rows span global ranges
  [1024t, 1024(t+1)) and need key-tiles tau < 8(t+1) -- identical structure on
  every core (SPMD-uniform), yet ~half the score/AV work of the full rectangle
  is skipped by causality. Diagonal masks are 8 host-provided 128x128 tiles.
- K/V projections sharded 8-way (rank r projects key-tiles {r,r+8,r+16,r+24});
  shared via 3 chip-wide AllGathers: K fp8 (4MB), V bf16 as two vd-halves
  (4MB each) so the vd0 A@V sweep overlaps the vd1 gather. The ~25-50us launch
  barrier is runtime-fixed (first collective executes ~58-85us in); the pre-AG
  window is filled with q-projection and a LOCAL attention pass on the core's
  own 4 key-tiles (kloc/vloc, no AG dependency). Own-rank stream tiles are
  zeroed via a per-tile exp bias input (-1e9), keeping the stream uniform.
- kvin stores ride the scalar queue (fire as soon as projection copies drain);
  gathered K/V stream back per RANK-block (512KB contiguous DMAs, 4KB lines).
- Scores run fp8 DoubleRow ([128,2,x] operand pairs over 256-wide contraction);
  exp folds the 1/sqrt(D) scale; softmax denominators ride the A@V sweep as
  16-wide ones-matmuls sharing the pt LDWEIGHTS; per-q-tile normalization is
  emitted inline at each accumulator's closing pair in the final rank.
- Projections and A@V stay bf16: fp8 V fails the 2e-2 gate (0.035 measured in
  simulation) and fp8 q/k-projection doubles the error for no critical-path
  gain. Measured: ~190-205us vs 318-331us baseline, rel err 0.0070.
"""

import numpy as np
import ml_dtypes

import concourse.bacc as bacc
import concourse.tile as tile
from concourse import mybir
from concourse.bass_utils import run_bass_kernel_spmd

S = 4096
D = 1024
NCORES = 8
P = 128
DC = 8          # contraction blocks of 128 in D
NT = 4          # q-tiles per core (128 rows each)
NKT = 32        # key tiles of 128 globally
BF = mybir.dt.bfloat16
F32 = mybir.dt.float32
FP8 = mybir.dt.float8e4
EXP = mybir.ActivationFunctionType.Exp
DR = mybir.MatmulPerfMode.DoubleRow
ALL8 = [[0, 1, 2, 3, 4, 5, 6, 7]]

bf16 = ml_dtypes.bfloat16


def build_nc(dbg=False):
    nc = bacc.Bacc(None, target_bir_lowering=False, debug=False)
    if dbg:
        d_pt = nc.declare_dram_parameter("d_pt", [4, P, 512], BF, isOutput=True)
        d_sums = nc.declare_dram_parameter("d_sums", [P, 64], F32, isOutput=True)

    xq = nc.declare_dram_parameter("xqT", [D, 512], BF, isOutput=False)
    xk = nc.declare_dram_parameter("xkT", [D, 512], BF, isOutput=False)
    xv = nc.declare_dram_parameter("xvT", [D, 512], BF, isOutput=False)
    wq = nc.declare_dram_parameter("wqT", [D, D], BF, isOutput=False)
    wk = nc.declare_dram_parameter("wkT", [D, D], BF, isOutput=False)
    wv = nc.declare_dram_parameter("wvT", [D, D], BF, isOutput=False)
    msk = nc.declare_dram_parameter("masks", [9, P, P], BF, isOutput=False)
    bias_p = nc.declare_dram_parameter("biasv", [P, NKT], F32, isOutput=False)
    out = nc.declare_dram_parameter("out", [512, D], F32, isOutput=True)

    kvin_k = nc.dram_tensor("kvin_k", [P, NT, DC, P], FP8)
    kvout_k = nc.dram_tensor("kvout_k", [NCORES * P, NT, DC, P], FP8, addr_space="Shared")
    VW = [640, 384]   # vd widths gathered per phase (640 = 512 + av5's 128)
    kvin_v = [nc.dram_tensor(f"kvin_v{h}", [P, NT, VW[h]], BF) for h in range(2)]
    kvout_v = [nc.dram_tensor(f"kvout_v{h}", [NCORES * P, NT, VW[h]], BF, addr_space="Shared")
               for h in range(2)]

    with tile.TileContext(nc) as tc:
        with (
            tc.tile_pool(name="persist", bufs=1) as persist,
            tc.tile_pool(name="wp", bufs=16) as wp,
            tc.tile_pool(name="xp", bufs=16) as xp,
            tc.tile_pool(name="ptp", bufs=1) as ptp,
            tc.tile_pool(name="ktp", bufs=4) as ktp,
            tc.tile_pool(name="vtp", bufs=4) as vtp,
            tc.tile_pool(name="outp", bufs=4) as outp,
            tc.tile_pool(name="sps", bufs=2, space="PSUM") as sps,
            tc.tile_pool(name="avs", bufs=1, space="PSUM") as avs,
            tc.tile_pool(name="sums", bufs=1, space="PSUM") as sums_pool,
        ):
            ones = persist.tile([P, 16], BF, tag="ones", name="ones")
            nc.vector.memset(ones[:], 1.0)

            m_t = [persist.tile([P, P], BF, tag=f"m{d}", name=f"m{d}") for d in range(9)]
            bias_t = persist.tile([P, NKT], F32, tag="bias", name="bias")

            qT = persist.tile([P, DC, 512], FP8, tag="qT", name="qT")
            kloc = persist.tile([P, NT, DC, P], FP8, tag="kloc", name="kloc")
            vloc = [persist.tile([P, NT, VW[h]], BF, tag=f"vloc{h}", name=f"vloc{h}")
                    for h in range(2)]

            # ---- K projection: kT blocks [outdim 128, 512 keys] -> fp8 ----
            wk_t = [wp.tile([P, D], BF, tag="w", name=f"wk{d}") for d in range(DC)]
            xk_t = [xp.tile([P, 512], BF, tag="x", name=f"xk{d}") for d in range(DC)]
            for d in range(DC):
                nc.sync.dma_start(out=xk_t[d][:], in_=xk[d * P:(d + 1) * P, :])
                nc.sync.dma_start(out=wk_t[d][:], in_=wk[d * P:(d + 1) * P, :])
            for ob in range(DC):
                ps = sps.tile([P, 512], F32, tag="sp", name="ppk")
                for d in range(DC):
                    nc.tensor.matmul(
                        ps[:], lhsT=wk_t[d][:, ob * P:(ob + 1) * P], rhs=xk_t[d][:],
                        start=(d == 0), stop=(d == DC - 1),
                    )
                for s in range(NT):
                    nc.scalar.copy(kloc[:, s, ob, :], ps[:, s * P:(s + 1) * P])

            # ---- V projection: v blocks [keys 128, 512 vd] -> bf16 ----
            wv_t = [wp.tile([P, D], BF, tag="w", name=f"wv{d}") for d in range(DC)]
            xv_t = [xp.tile([P, 512], BF, tag="x", name=f"xv{d}") for d in range(DC)]
            for d in range(DC):
                nc.sync.dma_start(out=xv_t[d][:], in_=xv[d * P:(d + 1) * P, :])
                nc.sync.dma_start(out=wv_t[d][:], in_=wv[d * P:(d + 1) * P, :])
            for h in range(2):
                for s in range(NT):
                    ps = sps.tile([P, 512], F32, tag="sp", name="ppv")
                    for d in range(DC):
                        nc.tensor.matmul(
                            ps[:], lhsT=xv_t[d][:, s * P:(s + 1) * P],
                            rhs=wv_t[d][:, h * 512:(h + 1) * 512],
                            start=(d == 0), stop=(d == DC - 1),
                        )
                    if h == 0:
                        nc.scalar.copy(vloc[0][:, s, 0:512], ps[:])
                    else:
                        nc.scalar.copy(vloc[0][:, s, 512:640], ps[:, 0:P])
                        nc.scalar.copy(vloc[1][:, s, :], ps[:, P:512])

            # ---- Q projection -> fp8 qT [128, cb, 512] ----
            wq_t = [wp.tile([P, D], BF, tag="w", name=f"wq{d}") for d in range(DC)]
            xq_t = [xp.tile([P, 512], BF, tag="x", name=f"xq{d}") for d in range(DC)]
            for d in range(DC):
                nc.sync.dma_start(out=xq_t[d][:], in_=xq[d * P:(d + 1) * P, :])
                nc.sync.dma_start(out=wq_t[d][:], in_=wq[d * P:(d + 1) * P, :])
            # constants arriving on sync after the big loads
            for dd in range(9):
                nc.sync.dma_start(out=m_t[dd][:], in_=msk[dd, :, :])
            nc.sync.dma_start(out=bias_t[:], in_=bias_p[:])
            for ob in range(DC):
                ps = sps.tile([P, 512], F32, tag="sp", name="ppq")
                for d in range(DC):
                    nc.tensor.matmul(
                        ps[:], lhsT=wq_t[d][:, ob * P:(ob + 1) * P], rhs=xq_t[d][:],
                        start=(d == 0), stop=(d == DC - 1),
                    )
                nc.scalar.copy(qT[:, ob, :], ps[:])

            # ---- collectives: triggers queue before the barrier ends ----
            # kvin stores ride the scalar queue: they fire right after the
            # projection copies drain instead of behind sync's 9MB of loads
            nc.scalar.dma_start(out=kvin_k[:], in_=kloc[:])
            nc.gpsimd.collective_compute(
                "AllGather", mybir.AluOpType.bypass, replica_groups=ALL8,
                ins=[kvin_k[:].opt()], outs=[kvout_k[:].opt()],
            )
            for h in range(2):
                nc.scalar.dma_start(out=kvin_v[h][:], in_=vloc[h][:])
                nc.gpsimd.collective_compute(
                    "AllGather", mybir.AluOpType.bypass, replica_groups=ALL8,
                    ins=[kvin_v[h][:].opt()], outs=[kvout_v[h][:].opt()],
                )

            sums_bank = sums_pool.tile([P, 64], F32, tag="sums", name="sums")
            av = {}
            for t in range(NT):
                av[t] = avs.tile([P, 512], F32, tag=f"av{t}", name=f"av{t}")
            # vd 512:640 accumulator: one bank, 4 q-tile regions (h0 only)
            av5 = avs.tile([P, 512], F32, tag="av5", name="av5")

            def scores_tile(kt_ap, rhs_hi, tmin, mask, bias_ap, ptag):
                """DR scores + exp + diag mask for one key tile; returns pt."""
                N = rhs_hi - tmin * P
                ps = sps.tile([P, 512], F32, tag="sp", name="sps")
                for dd in range(4):
                    nc.tensor.matmul(
                        ps[:, :N], lhsT=kt_ap[:, 2 * dd:2 * dd + 2, :],
                        rhs=qT[:, 2 * dd:2 * dd + 2, tmin * P:rhs_hi],
                        start=(dd == 0), stop=(dd == 3), perf_mode=DR,
                    )
                p = ptp.tile([P, 512], BF, tag=ptag, name=ptag)
                if bias_ap is None:
                    nc.scalar.activation(p[:, :N], ps[:, :N], EXP, scale=0.03125)
                else:
                    nc.scalar.activation(p[:, :N], ps[:, :N], EXP, scale=0.03125,
                                         bias=bias_ap)
                nc.vector.tensor_mul(p[:, 0:P], p[:, 0:P], mask[:])
                return p

            def sums_av(p, s, t, vt_ap, h, first, last):
                """piggybacked sums + AV MM(s) for pair (s, t); h0 adds av5"""
                pslice = p[:, (t - s) * P:(t - s + 1) * P]
                if h == 0:
                    nc.tensor.matmul(
                        sums_bank[:, t * 16:(t + 1) * 16], lhsT=pslice, rhs=ones[:],
                        start=first and t == 0, stop=last,
                        skip_group_check=True,
                    )
                    nc.tensor.matmul(
                        av[t][:, 0:512], lhsT=pslice, rhs=vt_ap[:, 0:512],
                        start=first, stop=last, skip_group_check=True,
                    )
                    nc.tensor.matmul(
                        av5[:, t * P:(t + 1) * P], lhsT=pslice, rhs=vt_ap[:, 512:640],
                        start=first and t == 0, stop=last, skip_group_check=True,
                    )
                else:
                    nc.tensor.matmul(
                        av[t][:, 0:384], lhsT=pslice, rhs=vt_ap,
                        start=first, stop=last, skip_group_check=True,
                    )

            # ---- local pass: own 4 key-tiles, no AG dependency ----
            ptl = {}
            for s in range(NT):
                ptl[s] = scores_tile(kloc[:, s, :, :], 512, s, m_t[8], None, f"ptl{s}")
                if dbg and s < 2:
                    nc.gpsimd.dma_start(out=d_pt[s, :, :], in_=ptl[s][:])
                for t in range(s, NT):
                    sums_av(ptl[s], s, t, vloc[0][:, s, :], 0, first=(s == 0),
                            last=False)

            # ---- stream scores, rank-major (own rank zeroed via bias) ----
            pt = {}
            ktb = {}
            for r in range(NCORES + 1):
                if r < NCORES:
                    kb = ktp.tile([P, NT, DC, P], FP8, tag="ktb", name="ktb")
                    ktb[r] = kb
                    nc.gpsimd.dma_start(out=kb[:], in_=kvout_k[r * P:(r + 1) * P, :, :, :])
                if r < 1:
                    continue
                rr = r - 1
                for s in range(NT):
                    tau = 8 * s + rr
                    pt[tau] = scores_tile(
                        ktb[rr][:, s, :, :], 512, s, m_t[rr],
                        bias_t[:, tau:tau + 1], f"pt{tau}")

            # ---- A@V vd-half sweeps, rank-major ----
            for h in range(2):
                if h == 1:
                    # re-alloc accumulators (banks freed after h=0 normalize)
                    for t in range(NT):
                        av[t] = avs.tile([P, 512], F32, tag=f"av{t}", name=f"av{t}")
                    # local pairs first: vloc-only, runs during AG_v1 wait
                    for s in range(NT):
                        for t in range(s, NT):
                            sums_av(ptl[s], s, t, vloc[1][:, s, :], 1,
                                    first=(s == 0), last=False)
                for r in range(NCORES):
                    vb = vtp.tile([P, NT, 640], BF, tag="vtb", name="vtb")
                    nc.gpsimd.dma_start(
                        out=vb[:, :, 0:VW[h]], in_=kvout_v[h][r * P:(r + 1) * P, :, :])
                    final = r == NCORES - 1
                    for s in range(NT):
                        tau = 8 * s + r
                        for t in range(s, NT):
                            # in the final rank, pair (s=t, t) is av[t]'s last MM:
                            # close the group and normalize inline
                            sums_av(pt[tau], s, t, vb[:, s, 0:VW[h]], h, first=False,
                                    last=(final and s == t))
                            if final and s == t:
                                ot = outp.tile([P, 640], F32, tag="ot", name="ot")
                                if h == 0:
                                    if dbg and t == NT - 1:
                                        dsb = outp.tile([P, 64], F32, tag="dsb", name="dsb")
                                        nc.vector.tensor_copy(dsb[:], sums_bank[:])
                                        nc.gpsimd.dma_start(out=d_sums[:], in_=dsb[:])
                                    rc = outp.tile([P, 1], F32, tag=f"rec{t}", name=f"rec{t}")
                                    if t == 0:
                                        rec = {}
                                    rec[t] = rc
                                    nc.vector.reciprocal(rc[:], sums_bank[:, t * 16:t * 16 + 1])
                                    nc.vector.tensor_scalar_mul(
                                        ot[:, 0:512], av[t][:], rec[t][:])
                                    nc.vector.tensor_scalar_mul(
                                        ot[:, 512:640], av5[:, t * P:(t + 1) * P], rec[t][:])
                                    nc.sync.dma_start(
                                        out=out[t * P:(t + 1) * P, 0:640], in_=ot[:])
                                else:
                                    nc.vector.tensor_scalar_mul(
                                        ot[:, 0:384], av[t][:, 0:384], rec[t][:])
                                    nc.sync.dma_start(
                                        out=out[t * P:(t + 1) * P, 640:1024],
                                        in_=ot[:, 0:384])
    return nc


_CACHE = {}


def _get_nc():
    if "nc" not in _CACHE:
        nc = build_nc()
        nc.compile()
        _CACHE["nc"] = nc
    return _CACHE["nc"]


def build_in_maps(inputs):
    x_q = np.asarray(inputs["encodings_for_q"], dtype=np.float32)
    x_k = np.asarray(inputs["encodings_for_k"], dtype=np.float32)
    x_v = np.asarray(inputs["encodings_for_v"], dtype=np.float32)
    W_q = np.asarray(inputs["W_q"], dtype=np.float32)
    W_k = np.asarray(inputs["W_k"], dtype=np.float32)
    W_v = np.asarray(inputs["W_v"], dtype=np.float32)

    # no 1/sqrt(D) folding: the exp activation applies scale=1/32
    wqT = np.ascontiguousarray(W_q.T).astype(bf16)
    wkT = np.ascontiguousarray(W_k.T).astype(bf16)
    wvT = np.ascontiguousarray(W_v.T).astype(bf16)

    ik = np.arange(P)[:, None]
    iq = np.arange(P)[None, :]

    in_maps = []
    for c in range(NCORES):
        rows = np.arange(c, S, NCORES)
        keys = np.concatenate([np.arange(P * (8 * s + c), P * (8 * s + c) + P)
                               for s in range(NT)])
        masks = np.stack([(128 * d + ik <= 8 * iq + c) for d in range(8)]
                         + [(128 * c + ik <= 8 * iq + c)])  # slot 8 = own diag (d=c)
        biasv = np.zeros((P, NKT), dtype=np.float32)
        biasv[:, [c, c + 8, c + 16, c + 24]] = -1e9   # zero own-rank stream tiles
        in_maps.append(dict(
            xqT=np.ascontiguousarray(x_q[rows].T).astype(bf16),
            xkT=np.ascontiguousarray(x_k[keys].T).astype(bf16),
            xvT=np.ascontiguousarray(x_v[keys].T).astype(bf16),
            wqT=wqT, wkT=wkT, wvT=wvT,
            masks=masks.astype(bf16),
            biasv=biasv,
        ))
    return in_maps


def kernel(**inputs):
    nc = _get_nc()
    in_maps = build_in_maps(inputs)
    res = run_bass_kernel_spmd(nc, in_maps, list(range(NCORES)))
    full = np.empty((S, D), dtype=np.float32)
    for c in range(NCORES):
        full[c::NCORES] = np.asarray(res.results[c]["out"], dtype=np.float32)
    return full
